# revision 1
# baseline (speedup 1.0000x reference)
"""NeRD pixel decoder (SIREN MLP over 5x5 local patches) on 8 trn2 cores.

Sharding: row-shard the pixel dim. Core c handles image b=c//4, rows
y0=(c%4)*32 .. y0+32 (4096 pixels). The 5x5 patch extraction is folded into
the first matmul as 25 shifted-window matmuls against a zero-padded slab
(rows with 2-halo, cols padded by 2 on each side). SIREN weights replicated.

All matmuls run in float32r (fp32 with 12-bit mantissa rounding) — full PE
rate at N=512. PSUM banks rotate 4-wide between consecutive matmuls to avoid
the same-bank accumulate hazard; sin() runs on the scalar engine draining one
half-phase while the PE fills the other.
"""

import numpy as np

FC = 128      # feature channels
P = 5         # patch
HID = 256
OUT = 3
OMEGA = 30.0
B, H, W = 2, 128, 128
NCORES = 8
ROWS = H // 4            # 32 image rows per core
NPIX = ROWS * W          # 4096 pixels per core
SLABR = ROWS + 4         # 36 slab rows (2 halo each side)
SLABW = W + 4            # 132 slab cols (2 pad each side)
TP = 512                 # pixels per PSUM tile (= 4 image rows)
NT = NPIX // TP          # 8 tiles per core
HB = 4                   # PSUM half-phase width (banks rotated per phase)

_BUILT = {}


def _build(structure="v2", loop_n=0, body="full"):
    """Build the per-core Bass program.

    structure: "v1" tile-major (per-tile accumulation), "v2" weight-stationary
    with 4-bank rotation. loop_n>0 wraps the body in a hardware loop for
    timing experiments (not used for the real kernel). body selects what goes
    inside the timing loop: full | dma | compute | empty.
    """
    key = (structure, loop_n, body)
    if key in _BUILT:
        return _BUILT[key]

    import concourse.tile as tile
    import concourse.mybir as mybir
    from concourse import bacc
    from contextlib import nullcontext

    f32 = mybir.dt.float32
    f32r = mybir.dt.float32r
    bf16 = mybir.dt.bfloat16
    Sin = mybir.ActivationFunctionType.Sin
    wdt = bf16 if structure == "v5" else f32r   # weight dtype

    nc = bacc.Bacc("TRN2", target_bir_lowering=False, debug=False)

    xs = nc.dram_tensor("xs", [128, SLABR * SLABW], f32r, kind="ExternalInput").ap()
    w0 = nc.dram_tensor("w0", [128, 25 * HID], wdt, kind="ExternalInput").ap()
    wc = nc.dram_tensor("wc", [2, HID], wdt, kind="ExternalInput").ap()
    co = nc.dram_tensor("co", [2, NPIX], f32r, kind="ExternalInput").ap()
    w1 = nc.dram_tensor("w1", [128, 4 * 128], wdt, kind="ExternalInput").ap()
    w2 = nc.dram_tensor("w2", [128, 4 * 128], wdt, kind="ExternalInput").ap()
    w3 = nc.dram_tensor("w3", [128, 2 * OUT], wdt, kind="ExternalInput").ap()
    b0 = nc.dram_tensor("b0", [128, 2], f32, kind="ExternalInput").ap()
    b1 = nc.dram_tensor("b1", [128, 2], f32, kind="ExternalInput").ap()
    b2 = nc.dram_tensor("b2", [128, 2], f32, kind="ExternalInput").ap()
    b3 = nc.dram_tensor("b3", [OUT, 1], f32, kind="ExternalInput").ap()
    out = nc.dram_tensor("out", [OUT, NPIX], f32, kind="ExternalOutput").ap()

    with tile.TileContext(nc) as tc:
        with (
            tc.tile_pool(name="const", bufs=1) as cpool,
            tc.tile_pool(name="h", bufs=2) as hpool,
            tc.tile_pool(name="osb", bufs=1) as opool,
            tc.tile_pool(name="ps", bufs=8, space="PSUM") as pspool,
        ):
            def emit_loads():
                T = {}
                T["xs_t"] = cpool.tile([128, SLABR * SLABW], f32r, tag="xs", name="xs_t")
                nc.sync.dma_start(T["xs_t"][:], xs[:])
                T["w0_t"] = cpool.tile([128, 25 * HID], wdt, tag="w0", name="w0_t")
                nc.sync.dma_start(T["w0_t"][:], w0[:])
                T["wc_t"] = cpool.tile([2, HID], wdt, tag="wc", name="wc_t")
                nc.sync.dma_start(T["wc_t"][:], wc[:])
                T["co_t"] = cpool.tile([2, NPIX], f32r, tag="co", name="co_t")
                nc.sync.dma_start(T["co_t"][:], co[:])
                T["w1_t"] = cpool.tile([128, 4 * 128], wdt, tag="w1", name="w1_t")
                nc.sync.dma_start(T["w1_t"][:], w1[:])
                T["w2_t"] = cpool.tile([128, 4 * 128], wdt, tag="w2", name="w2_t")
                nc.sync.dma_start(T["w2_t"][:], w2[:])
                T["w3_t"] = cpool.tile([128, 2 * OUT], wdt, tag="w3", name="w3_t")
                nc.sync.dma_start(T["w3_t"][:], w3[:])
                T["b0_t"] = cpool.tile([128, 2], f32, tag="b0", name="b0_t")
                nc.sync.dma_start(T["b0_t"][:], b0[:])
                T["b1_t"] = cpool.tile([128, 2], f32, tag="b1", name="b1_t")
                nc.sync.dma_start(T["b1_t"][:], b1[:])
                T["b2_t"] = cpool.tile([128, 2], f32, tag="b2", name="b2_t")
                nc.sync.dma_start(T["b2_t"][:], b2[:])
                T["b3_t"] = cpool.tile([OUT, 1], f32, tag="b3", name="b3_t")
                nc.sync.dma_start(T["b3_t"][:], b3[:])
                return T

            def emit_compute(T):
                xs_t, w0_t, wc_t, co_t = T["xs_t"], T["w0_t"], T["wc_t"], T["co_t"]
                w1_t, w2_t, w3_t = T["w1_t"], T["w2_t"], T["w3_t"]
                b0_t, b1_t, b2_t, b3_t = T["b0_t"], T["b1_t"], T["b2_t"], T["b3_t"]

                xs_r = xs_t[:].rearrange("p (r c) -> p r c", c=SLABW)

                def w0_chunk(o, m):
                    return w0_t[:, o * HID + m * 128: o * HID + m * 128 + 128]

                def rhs_l0(t, o):
                    dy, dx = divmod(o, 5)
                    return xs_r[:, 4 * t + dy: 4 * t + dy + 4, dx: dx + W]

                def h_slice(h, k, t):
                    return h[:, k * NPIX + t * TP: k * NPIX + (t + 1) * TP]

                # ---- layer 0: 5x5 conv (25 shifted matmuls) + coords, sin ----
                h0 = hpool.tile([128, 2 * NPIX], f32r, tag="h")
                if structure == "v3":
                    # software-pipelined: per m-phase, tile ti starts at round
                    # 4*ti; ends staggered 4 rounds apart so ACT drains never
                    # stall the PE; >=7-bank rotation between same-bank MMs.
                    STAG = 4
                    for m in range(2):
                        events = []
                        for ti in range(NT):
                            for o in range(26):
                                events.append((STAG * ti + o, ti, o))
                        events.sort()
                        pss = {}
                        for (_r, ti, o) in events:
                            if o == 0:
                                ps_i = pspool.tile([128, TP], f32, tag="ps",
                                                   name=f"ps_l0_{m}_{ti}")
                                pss[ti] = ps_i
                                nc.tensor.matmul(
                                    pss[ti][:], wc_t[:, m * 128:(m + 1) * 128],
                                    co_t[:, ti * TP:(ti + 1) * TP],
                                    start=True, stop=False)
                            else:
                                nc.tensor.matmul(
                                    pss[ti][:], w0_chunk(o - 1, m), rhs_l0(ti, o - 1),
                                    start=False, stop=(o == 25))
                            if o == 25:
                                nc.scalar.activation(
                                    h_slice(h0, m, ti), pss[ti][:], Sin,
                                    bias=b0_t[:, m:m + 1], scale=OMEGA)
                elif structure == "v1":
                    for t in range(NT):
                        for m in range(2):
                            ps = pspool.tile([128, TP], f32, tag="ps")
                            nc.tensor.matmul(
                                ps[:], wc_t[:, m * 128:(m + 1) * 128],
                                co_t[:, t * TP:(t + 1) * TP], start=True, stop=False)
                            for o in range(25):
                                nc.tensor.matmul(
                                    ps[:], w0_chunk(o, m), rhs_l0(t, o),
                                    start=False, stop=(o == 24))
                            nc.scalar.activation(
                                h_slice(h0, m, t), ps[:], Sin,
                                bias=b0_t[:, m:m + 1], scale=OMEGA)
                else:
                    for m in range(2):
                        for th in range(0, NT, HB):
                            pss = []
                            for _i in range(HB):
                                ps_i = pspool.tile([128, TP], f32, tag="ps")
                                pss.append(ps_i)
                            for ti in range(HB):
                                t = th + ti
                                nc.tensor.matmul(
                                    pss[ti][:], wc_t[:, m * 128:(m + 1) * 128],
                                    co_t[:, t * TP:(t + 1) * TP], start=True, stop=False)
                            for o in range(25):
                                for ti in range(HB):
                                    nc.tensor.matmul(
                                        pss[ti][:], w0_chunk(o, m), rhs_l0(th + ti, o),
                                        start=False, stop=(o == 24))
                            for ti in range(HB):
                                nc.scalar.activation(
                                    h_slice(h0, m, th + ti), pss[ti][:], Sin,
                                    bias=b0_t[:, m:m + 1], scale=OMEGA)

                # ---- layers 1, 2: dense 256->256, sin ----
                hin = h0
                for (wl_t, bl_t) in ((w1_t, b1_t), (w2_t, b2_t)):
                    hout = hpool.tile([128, 2 * NPIX], f32r, tag="h")
                    if structure == "v3":
                        # k-major over all (m, ti): 16 groups x 2 chunks,
                        # 8-bank rotation (bank reused 8 groups later).
                        for half in range(2):       # tiles 0-3 + m pairs, then 4-7
                            groups = [(m, th) for th in range(half * 4, half * 4 + 4)
                                      for m in range(2)]
                            pss = {}
                            for gi, (m, t) in enumerate(groups):
                                ps_i = pspool.tile([128, TP], f32, tag="ps",
                                                   name=f"ps_l12_{half}_{gi}")
                                pss[(m, t)] = ps_i
                            for k in range(2):
                                for (m, t) in groups:
                                    nc.tensor.matmul(
                                        pss[(m, t)][:],
                                        wl_t[:, (k * 2 + m) * 128:(k * 2 + m + 1) * 128],
                                        h_slice(hin, k, t),
                                        start=(k == 0), stop=(k == 1))
                            for (m, t) in groups:
                                nc.scalar.activation(
                                    h_slice(hout, m, t), pss[(m, t)][:], Sin,
                                    bias=bl_t[:, m:m + 1], scale=OMEGA)
                    elif structure == "v1":
                        for t in range(NT):
                            for m in range(2):
                                ps = pspool.tile([128, TP], f32, tag="ps")
                                for k in range(2):
                                    nc.tensor.matmul(
                                        ps[:], wl_t[:, (k * 2 + m) * 128:(k * 2 + m + 1) * 128],
                                        h_slice(hin, k, t), start=(k == 0), stop=(k == 1))
                                nc.scalar.activation(
                                    h_slice(hout, m, t), ps[:], Sin,
                                    bias=bl_t[:, m:m + 1], scale=OMEGA)
                    else:
                        for m in range(2):
                            for th in range(0, NT, HB):
                                pss = []
                                for _i in range(HB):
                                    ps_i = pspool.tile([128, TP], f32, tag="ps")
                                    pss.append(ps_i)
                                for k in range(2):
                                    for ti in range(HB):
                                        nc.tensor.matmul(
                                            pss[ti][:],
                                            wl_t[:, (k * 2 + m) * 128:(k * 2 + m + 1) * 128],
                                            h_slice(hin, k, th + ti),
                                            start=(k == 0), stop=(k == 1))
                                for ti in range(HB):
                                    nc.scalar.activation(
                                        h_slice(hout, m, th + ti), pss[ti][:], Sin,
                                        bias=bl_t[:, m:m + 1], scale=OMEGA)
                    hin = hout

                # ---- head: 256 -> 3, + bias ----
                out_sb = opool.tile([OUT, NPIX], f32, tag="osb")
                HBH = NT if structure == "v3" else HB
                for th in range(0, NT, HBH):
                    pss = []
                    for _i in range(HBH):
                        ps_i = pspool.tile([OUT, TP], f32, tag="ps")
                        pss.append(ps_i)
                    for k in range(2):
                        for ti in range(HBH):
                            nc.tensor.matmul(
                                pss[ti][:], w3_t[:, k * OUT:(k + 1) * OUT],
                                h_slice(hin, k, th + ti), start=(k == 0), stop=(k == 1))
                    for ti in range(HBH):
                        t = th + ti
                        nc.vector.tensor_scalar_add(
                            out_sb[:, t * TP:(t + 1) * TP], pss[ti][:], b3_t[:, 0:1])
                nc.sync.dma_start(out[:], out_sb[:])

            loop_cm = (
                tc.For_i(0, loop_n, 1, hint_engines=(mybir.EngineType.PE,))
                if loop_n else nullcontext()
            )
            T_pre = emit_loads() if body == "compute" else None
            with loop_cm:
                if body == "full":
                    emit_compute(emit_loads())
                elif body == "compute":
                    emit_compute(T_pre)
                elif body == "dma":
                    T = emit_loads()
                    ob = opool.tile([OUT, NPIX], f32, tag="osb")
                    nc.scalar.mul(ob[:, 0:1], T["b3_t"][:, 0:1], 1.0)
                    nc.sync.dma_start(out[:, 0:1], ob[:, 0:1])
                elif body == "empty":
                    ob = opool.tile([OUT, NPIX], f32, tag="osb")
                    nc.vector.memset(ob[:, 0:1], 0.0)
                    nc.sync.dma_start(out[:, 0:1], ob[:, 0:1])

    nc.finalize()
    _BUILT[key] = nc
    return nc


def _to_f32r(a):
    """Round fp32 to the fp32r format the PE expects (low 12 mantissa bits 0)."""
    b = np.ascontiguousarray(a, np.float32).view(np.uint32).astype(np.uint64)
    r = ((b + 0x800) & 0xFFFFF000).astype(np.uint32)
    return r.view(np.float32).reshape(np.asarray(a).shape)


def _prep_core_inputs(c, xi, coords_full):
    b = c // 4
    y0 = (c % 4) * ROWS
    slab = np.zeros((128, SLABR, SLABW), np.float32)
    ylo, yhi = y0 - 2, y0 + ROWS + 2
    slo, shi = max(ylo, 0), min(yhi, H)
    slab[:, slo - ylo: shi - ylo, 2:2 + W] = xi[b, :, slo:shi, :]

    co = coords_full[:, y0 * W:(y0 + ROWS) * W]

    return {
        "xs": _to_f32r(slab.reshape(128, SLABR * SLABW)),
        "co": _to_f32r(co),
    }


def kernel(**inputs):
    from concourse.bass_utils import run_bass_kernel_spmd

    xi = np.asarray(inputs["xi"], np.float32)
    W0 = np.asarray(inputs["W0"], np.float32)
    b0 = np.asarray(inputs["b0"], np.float32)
    W1 = np.asarray(inputs["W1"], np.float32)
    b1 = np.asarray(inputs["b1"], np.float32)
    W2 = np.asarray(inputs["W2"], np.float32)
    b2 = np.asarray(inputs["b2"], np.float32)
    W3 = np.asarray(inputs["W3"], np.float32)
    b3 = np.asarray(inputs["b3"], np.float32)

    # replicated weight tensors, rearranged for the PE (lhsT chunks)
    w0_h = _to_f32r(W0[:FC * P * P].reshape(128, 25 * HID))
    wc_h = _to_f32r(W0[FC * P * P:])                       # [2, 256]
    w1_h = _to_f32r(
        W1.reshape(2, 128, 2, 128).transpose(1, 0, 2, 3).reshape(128, 512))
    w2_h = _to_f32r(
        W2.reshape(2, 128, 2, 128).transpose(1, 0, 2, 3).reshape(128, 512))
    w3_h = _to_f32r(
        W3.reshape(2, 128, OUT).transpose(1, 0, 2).reshape(128, 2 * OUT))
    b0_h = np.ascontiguousarray((OMEGA * b0).reshape(2, 128).T)
    b1_h = np.ascontiguousarray((OMEGA * b1).reshape(2, 128).T)
    b2_h = np.ascontiguousarray((OMEGA * b2).reshape(2, 128).T)
    b3_h = np.ascontiguousarray(b3.reshape(OUT, 1))

    # normalized coords, matching jnp.linspace/meshgrid in the reference
    ys = np.linspace(-1.0, 1.0, H, dtype=np.float32)
    xcs = np.linspace(-1.0, 1.0, W, dtype=np.float32)
    gy, gx = np.meshgrid(ys, xcs, indexing="ij")
    coords_full = np.stack([gx.reshape(-1), gy.reshape(-1)], 0).astype(np.float32)

    shared = {
        "w0": w0_h, "wc": wc_h, "w1": w1_h, "w2": w2_h, "w3": w3_h,
        "b0": b0_h, "b1": b1_h, "b2": b2_h, "b3": b3_h,
    }
    in_maps = []
    for c in range(NCORES):
        m = _prep_core_inputs(c, xi, coords_full)
        m.update(shared)
        in_maps.append(m)

    nc = _build()
    res = run_bass_kernel_spmd(nc, in_maps, core_ids=list(range(NCORES)))

    full = np.empty((B, OUT, H, W), np.float32)
    for c in range(NCORES):
        b = c // 4
        y0 = (c % 4) * ROWS
        full[b, :, y0:y0 + ROWS, :] = res.results[c]["out"].reshape(OUT, ROWS, W)
    return full



# revision 7
# speedup vs baseline: 1.1317x; 1.1317x over previous
"""NeRD pixel decoder (SIREN MLP over 5x5 local patches) on 8 trn2 cores.

Sharding: row-shard the pixel dim. Core c handles image b=c//4, rows
y0=(c%4)*32 .. y0+32 (4096 pixels). The 5x5 patch extraction is folded into
the first matmul as 25 shifted-window matmuls against a zero-padded slab
(rows with 2-halo, cols padded by 2 on each side). SIREN weights replicated.

All matmul operands are bf16 (PSUM accumulation stays fp32): same PE rate as
fp32r at N=512 but half the DMA/SBUF traffic. Input DMAs are split into
consumption-order chunks and spread over the SP/Pool/DVE queues so the first
matmuls start ~1us in (vs ~24us for monolithic loads); a few ap=128 warmup
matmuls ramp the PE DVFS p-state while the first chunks land. PSUM banks
rotate 4-wide between consecutive matmuls to avoid the same-bank accumulate
hazard; sin() drains on the scalar engine while the PE fills the other
half-phase. The 256->3 head runs per 4-tile group with bias-adds interleaved
on DVE/Pool and the output DMA streamed per group.
"""

import numpy as np

FC = 128      # feature channels
P = 5         # patch
HID = 256
OUT = 3
OMEGA = 30.0
B, H, W = 2, 128, 128
NCORES = 8
ROWS = H // 4            # 32 image rows per core
NPIX = ROWS * W          # 4096 pixels per core
SLABR = ROWS + 4         # 36 slab rows (2 halo each side)
SLABW = W + 4            # 132 slab cols (2 pad each side)
TP = 512                 # pixels per PSUM tile (= 4 image rows)
NT = NPIX // TP          # 8 tiles per core
HB = 4                   # PSUM half-phase width (banks rotated per phase)
NWARM = 6                # warmup matmuls (p-state ramp while DMAs land)

_BUILT = {}


def _build():
    import concourse.tile as tile
    import concourse.mybir as mybir
    from concourse import bacc

    if "nc" in _BUILT:
        return _BUILT["nc"]

    f32 = mybir.dt.float32
    bf16 = mybir.dt.bfloat16
    Sin = mybir.ActivationFunctionType.Sin
    Identity = mybir.ActivationFunctionType.Identity

    nc = bacc.Bacc("TRN2", target_bir_lowering=False, debug=False)

    xs = nc.dram_tensor("xs", [128, SLABR * SLABW], bf16, kind="ExternalInput").ap()
    w0a = nc.dram_tensor("w0a", [128, 25 * 128], bf16, kind="ExternalInput").ap()
    w0b = nc.dram_tensor("w0b", [128, 25 * 128], bf16, kind="ExternalInput").ap()
    cowc = nc.dram_tensor("cowc", [2, NPIX + HID], bf16, kind="ExternalInput").ap()
    w123 = nc.dram_tensor("w123", [128, 8 * 128 + 2 * OUT], bf16,
                          kind="ExternalInput").ap()
    b012 = nc.dram_tensor("b012", [128, 6], f32, kind="ExternalInput").ap()
    b3 = nc.dram_tensor("b3", [OUT, 1], f32, kind="ExternalInput").ap()
    out = nc.dram_tensor("out", [OUT, NPIX], f32, kind="ExternalOutput").ap()

    RW = SLABW

    with tile.TileContext(nc) as tc:
        with (
            tc.tile_pool(name="const", bufs=1) as cpool,
            tc.tile_pool(name="h", bufs=2) as hpool,
            tc.tile_pool(name="osb", bufs=1) as opool,
            tc.tile_pool(name="ps", bufs=8, space="PSUM") as pspool,
        ):
            # ---- loads: consumption-order chunks over 3 DMA queues ----
            xs_t = cpool.tile([128, SLABR * SLABW], bf16, tag="xs", name="xs_t")
            w0a_t = cpool.tile([128, 25 * 128], bf16, tag="w0a", name="w0a_t")
            w0b_t = cpool.tile([128, 25 * 128], bf16, tag="w0b", name="w0b_t")
            cowc_t = cpool.tile([2, NPIX + HID], bf16, tag="cowc", name="cowc_t")
            w123_t = cpool.tile([128, 8 * 128 + 2 * OUT], bf16, tag="w123",
                                name="w123_t")
            b012_t = cpool.tile([128, 6], f32, tag="b012", name="b012_t")
            b3_t = cpool.tile([OUT, 1], f32, tag="b3", name="b3_t")
            wz_t = cpool.tile([128, 128], bf16, tag="wz", name="wz_t")

            # SP queue: critical path for the first tile groups
            nc.sync.dma_start(cowc_t[:], cowc[:])
            nc.sync.dma_start(xs_t[:, 0:12 * RW], xs[:, 0:12 * RW])
            nc.sync.dma_start(xs_t[:, 12 * RW:24 * RW], xs[:, 12 * RW:24 * RW])
            nc.sync.dma_start(b012_t[:], b012[:])
            # Pool queue: w0 (m=0) in o-order chunks, then the xs tail
            nc.gpsimd.memset(wz_t[:], 0.0)
            nc.gpsimd.dma_start(w0a_t[:, 0:5 * 128], w0a[:, 0:5 * 128])
            nc.gpsimd.dma_start(w0a_t[:, 5 * 128:15 * 128], w0a[:, 5 * 128:15 * 128])
            nc.gpsimd.dma_start(w0a_t[:, 15 * 128:25 * 128], w0a[:, 15 * 128:25 * 128])
            nc.gpsimd.dma_start(xs_t[:, 24 * RW:36 * RW], xs[:, 24 * RW:36 * RW])
            # Activation queue: w0 (m=1) + later layers (issued before any sins)
            nc.scalar.dma_start(w0b_t[:], w0b[:])
            nc.scalar.dma_start(w123_t[:], w123[:])
            nc.scalar.dma_start(b3_t[:], b3[:])

            co_t = cowc_t[:, 0:NPIX]
            wc_t = cowc_t[:, NPIX:NPIX + HID]
            xs_r = xs_t[:].rearrange("p (r c) -> p r c", c=SLABW)

            def w0_chunk(m, o):
                t = w0a_t if m == 0 else w0b_t
                return t[:, o * 128:(o + 1) * 128]

            def rhs_l0(t, o):
                dy, dx = divmod(o, 5)
                return xs_r[:, 4 * t + dy: 4 * t + dy + 4, dx: dx + W]

            def h_slice(h, k, t):
                return h[:, k * NPIX + t * TP: k * NPIX + (t + 1) * TP]

            # ---- warmup: ramp the PE p-state on zeros while DMAs stream ----
            for _ in range(NWARM):
                wps = pspool.tile([128, 128], f32, tag="ps", name="ps_warm")
                nc.tensor.matmul(wps[:], wz_t[:], wz_t[:], start=True, stop=True)

            # ---- layer 0: 5x5 conv (25 shifted matmuls) + coords, sin ----
            h0 = hpool.tile([128, 2 * NPIX], bf16, tag="h", name="h0")
            for m in range(2):
                for th in (0, HB):
                    pss = [pspool.tile([128, TP], f32, tag="ps", name=f"ps_{m}_{th}_{i}") for i in range(HB)]
                    for ti in range(HB):
                        t = th + ti
                        nc.tensor.matmul(
                            pss[ti][:], wc_t[:, m * 128:(m + 1) * 128],
                            co_t[:, t * TP:(t + 1) * TP], start=True, stop=False)
                    for o in range(25):
                        for ti in range(HB):
                            nc.tensor.matmul(
                                pss[ti][:], w0_chunk(m, o), rhs_l0(th + ti, o),
                                start=False, stop=(o == 24))
                    for ti in range(HB):
                        nc.scalar.activation(
                            h_slice(h0, m, th + ti), pss[ti][:], Sin,
                            bias=b012_t[:, m:m + 1], scale=OMEGA)

            # ---- layers 1, 2: dense 256->256, sin ----
            hin = h0
            for li in range(2):
                bl_t = b012_t[:, 2 + 2 * li: 4 + 2 * li]
                wl_t = w123_t[:, li * 4 * 128:(li + 1) * 4 * 128]
                hout = hpool.tile([128, 2 * NPIX], bf16, tag="h", name=f"h{li+1}")
                for m in range(2):
                    for th in (0, HB):
                        pss = [pspool.tile([128, TP], f32, tag="ps",
                                          name=f"ps_l{li}_{m}_{th}_{i}")
                               for i in range(HB)]
                        for k in range(2):
                            for ti in range(HB):
                                nc.tensor.matmul(
                                    pss[ti][:],
                                    wl_t[:, (k * 2 + m) * 128:(k * 2 + m + 1) * 128],
                                    h_slice(hin, k, th + ti),
                                    start=(k == 0), stop=(k == 1))
                        for ti in range(HB):
                            nc.scalar.activation(
                                h_slice(hout, m, th + ti), pss[ti][:], Sin,
                                bias=bl_t[:, m:m + 1], scale=OMEGA)
                hin = hout

            # ---- head: 256 -> 3, + bias, streamed out per 4-tile group ----
            w3_t = w123_t[:, 8 * 128:8 * 128 + 2 * OUT]
            out_sb = opool.tile([OUT, NPIX], f32, tag="osb", name="out_sb")
            for th in (0, HB):
                pss = [pspool.tile([OUT, TP], f32, tag="ps", name=f"ps_hd_{th}_{i}") for i in range(HB)]
                for k in range(2):
                    for ti in range(HB):
                        t = th + ti
                        nc.tensor.matmul(
                            pss[ti][:], w3_t[:, k * OUT:(k + 1) * OUT],
                            h_slice(hin, k, t), start=(k == 0), stop=(k == 1))
                        if k == 1:
                            if ti < 2:
                                nc.vector.tensor_scalar_add(
                                    out_sb[:, t * TP:(t + 1) * TP], pss[ti][:],
                                    b3_t[:, 0:1])
                            else:
                                nc.scalar.activation(
                                    out_sb[:, t * TP:(t + 1) * TP], pss[ti][:],
                                    Identity, bias=b3_t[:, 0:1], scale=1.0)
                nc.sync.dma_start(out[:, th * TP:(th + HB) * TP],
                                  out_sb[:, th * TP:(th + HB) * TP])

    nc.finalize()
    _BUILT["nc"] = nc
    return nc


def _prep_core_inputs(c, xi_bf, coords_full):
    b = c // 4
    y0 = (c % 4) * ROWS
    import ml_dtypes
    slab = np.zeros((128, SLABR, SLABW), ml_dtypes.bfloat16)
    ylo, yhi = y0 - 2, y0 + ROWS + 2
    slo, shi = max(ylo, 0), min(yhi, H)
    slab[:, slo - ylo: shi - ylo, 2:2 + W] = xi_bf[b, :, slo:shi, :]

    co = coords_full[:, y0 * W:(y0 + ROWS) * W]
    return {"xs": slab.reshape(128, SLABR * SLABW), "co": co}


def kernel(**inputs):
    import ml_dtypes
    from concourse.bass_utils import run_bass_kernel_spmd

    bf = ml_dtypes.bfloat16
    xi = np.asarray(inputs["xi"], np.float32)
    W0 = np.asarray(inputs["W0"], np.float32)
    b0 = np.asarray(inputs["b0"], np.float32)
    W1 = np.asarray(inputs["W1"], np.float32)
    b1 = np.asarray(inputs["b1"], np.float32)
    W2 = np.asarray(inputs["W2"], np.float32)
    b2 = np.asarray(inputs["b2"], np.float32)
    W3 = np.asarray(inputs["W3"], np.float32)
    b3 = np.asarray(inputs["b3"], np.float32)

    # replicated weight tensors, rearranged for the PE (lhsT chunks)
    w0p = W0[:FC * P * P].reshape(128, 25, HID)          # [c, o, j]
    w0a_h = np.ascontiguousarray(w0p[:, :, :128]).reshape(128, 25 * 128).astype(bf)
    w0b_h = np.ascontiguousarray(w0p[:, :, 128:]).reshape(128, 25 * 128).astype(bf)
    wc_h = W0[FC * P * P:]                               # [2, 256]
    w1_h = W1.reshape(2, 128, 2, 128).transpose(1, 0, 2, 3).reshape(128, 512)
    w2_h = W2.reshape(2, 128, 2, 128).transpose(1, 0, 2, 3).reshape(128, 512)
    w3_h = W3.reshape(2, 128, OUT).transpose(1, 0, 2).reshape(128, 2 * OUT)
    w123_h = np.concatenate([w1_h, w2_h, w3_h], axis=1).astype(bf)
    b012_h = np.ascontiguousarray(np.concatenate(
        [(OMEGA * b).reshape(2, 128).T for b in (b0, b1, b2)], axis=1))
    b3_h = np.ascontiguousarray(b3.reshape(OUT, 1))

    # normalized coords, matching jnp.linspace/meshgrid in the reference
    ys = np.linspace(-1.0, 1.0, H, dtype=np.float32)
    xcs = np.linspace(-1.0, 1.0, W, dtype=np.float32)
    gy, gx = np.meshgrid(ys, xcs, indexing="ij")
    coords_full = np.stack([gx.reshape(-1), gy.reshape(-1)], 0).astype(bf)

    xi_bf = xi.astype(bf)

    shared = {
        "w0a": w0a_h, "w0b": w0b_h, "w123": w123_h,
        "b012": b012_h, "b3": b3_h,
    }
    in_maps = []
    for c in range(NCORES):
        m = _prep_core_inputs(c, xi_bf, coords_full)
        m["cowc"] = np.concatenate([m.pop("co"), wc_h.astype(bf)], axis=1)
        m.update(shared)
        in_maps.append(m)

    nc = _build()
    res = run_bass_kernel_spmd(nc, in_maps, core_ids=list(range(NCORES)))

    full = np.empty((B, OUT, H, W), np.float32)
    for c in range(NCORES):
        b = c // 4
        y0 = (c % 4) * ROWS
        full[b, :, y0:y0 + ROWS, :] = res.results[c]["out"].reshape(OUT, ROWS, W)
    return full
